# revision 2
# baseline (speedup 1.0000x reference)
"""Causal self-attention (B=2, T=2048, C=1024, H=16, D=64) on 8 TRN2 cores.

Sharding: core = b*4 + g handles batch b, heads 4g..4g+3 (data parallel on B,
tensor parallel on heads). Each core computes its 4 heads' contribution to
x @ W_proj; host sums the 4 partial outputs per batch and adds b_proj.

On-core pipeline (all matmuls float32r ~= tf32-ish, l2err ~1.5e-4):
  A) qkT = Wqk.T @ xT (feat-major [128,T] tiles, 2 heads/chunk) and
     v = xT.T @ Wv ([T,256] t-tiles, ones-augmented per head for the
     softmax denominator trick), RoPE applied to q/k in [d,T] layout.
  B) flash-style causal attention per head-pair / q-chunk:
     sT = kT.T @ qT (K=64 matmuls, 2 heads packed via tile_position),
     exp via ACT (scale=1/8) PSUM->SBUF, causal mask TT-mul on diagonal
     tiles, yT[65,Tq] += v_aug.T @ exp(sT); denominator = ones row.
     Normalize: recip = exp(-ln(den)) on ACT, K=1 matmul broadcast,
     TT mul -> yT sbuf chunks [128, T].
  C) out[t, :] = yT.T @ Wproj_slice, accumulated over 2 c_local chunks.
"""
import os
import numpy as np

import concourse.bass as bass
import concourse.mybir as mybir
from concourse import bacc
from concourse.tile import TileContext
from concourse.bass_utils import run_bass_kernel_spmd

B, T, C, H, D = 2, 2048, 1024, 16, 64
HPC = 4          # heads per core
NCORES = 8
TCH = 512        # t-chunk / q-chunk width
NTC = T // TCH   # 4
NTT = T // 128   # 16 t-tiles
NCC = C // 128   # 8 c-chunks
F32R = mybir.dt.float32r
F32 = mybir.dt.float32
BF16 = mybir.dt.bfloat16
MMDT = BF16  # dtype for matmul operands (PSUM accum stays fp32)
AF = mybir.ActivationFunctionType
ALU = mybir.AluOpType

_prog_cache = {}
_DEBUG_DUMPS = False


def _build_program(has_battn: bool):
    nc = bacc.Bacc("TRN2", target_bir_lowering=False, debug=False,
                   num_devices=NCORES)
    # ---- DRAM I/O (per core) ----
    xT_d = nc.dram_tensor("xT", [C, T], MMDT, kind="ExternalInput")
    wqk_d = nc.dram_tensor("wqk", [C, 4 * 128], MMDT, kind="ExternalInput")
    wv_d = nc.dram_tensor("wv", [C, HPC * D], MMDT, kind="ExternalInput")
    wp_d = nc.dram_tensor("wp", [HPC * D, C], MMDT, kind="ExternalInput")
    cos_d = nc.dram_tensor("cos_t", [128, T], F32, kind="ExternalInput")
    sin_d = nc.dram_tensor("sin_t", [128, T], F32, kind="ExternalInput")
    mask_d = nc.dram_tensor("masks", [4 * 128, TCH], MMDT, kind="ExternalInput")
    ones4_d = nc.dram_tensor("ones4", [128, HPC], MMDT, kind="ExternalInput")
    ones1_d = nc.dram_tensor("ones1", [1, D], MMDT, kind="ExternalInput")
    bqk_d = nc.dram_tensor("bqk", [4 * 128, 1], F32, kind="ExternalInput")
    vbias_d = nc.dram_tensor("vbias", [128, HPC * D], MMDT, kind="ExternalInput")
    out_d = nc.dram_tensor("out", [T, C], F32, kind="ExternalOutput")
    if _DEBUG_DUMPS:
        dq_d = nc.dram_tensor("dbg_qT0", [128, T], F32, kind="ExternalOutput")
        dk_d = nc.dram_tensor("dbg_kT0", [128, T], F32, kind="ExternalOutput")
        dy_d = nc.dram_tensor("dbg_yT0", [128, T], F32, kind="ExternalOutput")
        dv_d = nc.dram_tensor("dbg_va0", [128, HPC * (D + 1)], F32, kind="ExternalOutput")

    with TileContext(nc) as tc:
        with (
            tc.tile_pool(name="wsb", bufs=1) as wsb,      # persistent weights/tables
            tc.tile_pool(name="xsb", bufs=16) as xsb,     # streamed xT tiles
            tc.tile_pool(name="qk", bufs=1) as qksb,      # persistent qT/kT/yT/v
            tc.tile_pool(name="esb", bufs=8) as esb,      # exp tiles
            tc.tile_pool(name="osb", bufs=2) as osb,      # out staging + small
            tc.tile_pool(name="ps", bufs=2, space="PSUM") as ps,
        ):
            # ---- persistent loads ----
            wqk_sb = []
            for cc in range(NCC):
                t_ = wsb.tile([128, 4 * 128], MMDT, tag=f"wqk{cc}", name=f"wqk{cc}")
                nc.sync.dma_start(out=t_[:], in_=wqk_d[cc * 128:(cc + 1) * 128, :])
                wqk_sb.append(t_)
            wv_sb = []
            for cc in range(NCC):
                t_ = wsb.tile([128, HPC * D], MMDT, tag=f"wv{cc}", name=f"wv{cc}")
                nc.sync.dma_start(out=t_[:], in_=wv_d[cc * 128:(cc + 1) * 128, :])
                wv_sb.append(t_)
            wp_sb = []
            for kk in range(2):
                t_ = wsb.tile([128, C], MMDT, tag=f"wp{kk}", name=f"wp{kk}")
                nc.sync.dma_start(out=t_[:], in_=wp_d[kk * 128:(kk + 1) * 128, :])
                wp_sb.append(t_)
            cos_sb = wsb.tile([128, T], F32, tag="cos")
            sin_sb = wsb.tile([128, T], F32, tag="sin")
            nc.sync.dma_start(out=cos_sb[:], in_=cos_d[:])
            nc.sync.dma_start(out=sin_sb[:], in_=sin_d[:])
            mask_sb = []
            for m in range(4):
                t_ = wsb.tile([128, TCH], MMDT, tag=f"mask{m}", name=f"mask{m}")
                nc.sync.dma_start(out=t_[:], in_=mask_d[m * 128:(m + 1) * 128, :])
                mask_sb.append(t_)
            ones1_sb = wsb.tile([1, D], MMDT, tag="ones1")
            nc.sync.dma_start(out=ones1_sb[:], in_=ones1_d[:])
            bqk_sb = wsb.tile([4 * 128, 1], F32, tag="bqk") if has_battn else None
            if has_battn:
                bqk_sb = []
                for ft in range(4):
                    t_ = wsb.tile([128, 1], F32, tag=f"bqk{ft}", name=f"bqk{ft}")
                    nc.sync.dma_start(out=t_[:], in_=bqk_d[ft * 128:(ft + 1) * 128, :])
                    bqk_sb.append(t_)
                vbias_sb = wsb.tile([128, HPC * D], MMDT, tag="vbias")
                nc.sync.dma_start(out=vbias_sb[:], in_=vbias_d[:])

            # persistent activations
            qT = [qksb.tile([128, T], MMDT, tag=f"qT{p}", name=f"qT{p}") for p in range(2)]
            kT = [qksb.tile([128, T], MMDT, tag=f"kT{p}", name=f"kT{p}") for p in range(2)]
            yT = [qksb.tile([128, T], MMDT, tag=f"yT{p}", name=f"yT{p}") for p in range(2)]
            vaug = [qksb.tile([128, HPC * (D + 1)], MMDT, tag=f"va{tt}", name=f"va{tt}")
                    for tt in range(NTT)]
            # ones columns of v_aug via one strided DMA per tile
            for tt in range(NTT):
                nc.sync.dma_start(
                    out=vaug[tt][:, D::D + 1],  # cols D, 2D+1, ... (h*(D+1)+D)
                    in_=ones4_d[:])

            # ---- Phase A: qkv projection ----
            for tci in range(NTC):
                xt = []
                for cc in range(NCC):
                    t_ = xsb.tile([128, TCH], MMDT, tag="xt", name=f"xt_{tci}_{cc}")
                    nc.sync.dma_start(
                        out=t_[:],
                        in_=xT_d[cc * 128:(cc + 1) * 128,
                                 tci * TCH:(tci + 1) * TCH])
                    xt.append(t_)
                # qkT: out [feat 128, TCH] per feat tile
                qk_dst = [qT[0], qT[1], kT[0], kT[1]]
                for ft in range(4):
                    pqk = ps.tile([128, TCH], F32, tag="p512", name=f"pqk_{tci}_{ft}")
                    for cc in range(NCC):
                        nc.tensor.matmul(
                            pqk[:], wqk_sb[cc][:, ft * 128:(ft + 1) * 128],
                            xt[cc][:], start=(cc == 0), stop=(cc == NCC - 1))
                    dst = qk_dst[ft][:, tci * TCH:(tci + 1) * TCH]
                    if has_battn:
                        nc.scalar.activation(dst, pqk[:], AF.Identity,
                                             bias=bqk_sb[ft][:])
                    else:
                        nc.scalar.copy(dst, pqk[:])
                # v: out [t 128, 256] per t-tile
                for j in range(4):
                    tt = tci * 4 + j
                    pv = ps.tile([128, HPC * D], F32, tag="acc", name=f"pv_{tt}")
                    for cc in range(NCC):
                        nc.tensor.matmul(
                            pv[:], xt[cc][:, j * 128:(j + 1) * 128],
                            wv_sb[cc][:], start=(cc == 0), stop=(cc == NCC - 1))
                    # write into interleaved v_aug cols h*(D+1)+d
                    dst = vaug[tt][:, 0:HPC * (D + 1)].rearrange(
                        "p (h e) -> p h e", e=D + 1)[:, :, 0:D]
                    if has_battn:
                        nc.vector.scalar_tensor_tensor(
                            dst, pv[:].rearrange("p (h e) -> p h e", e=D),
                            0.0, vbias_sb[:].rearrange("p (h e) -> p h e", e=D),
                            ALU.add, ALU.add)
                    else:
                        nc.scalar.copy(
                            dst, pv[:].rearrange("p (h e) -> p h e", e=D))

            # ---- RoPE on qT, kT (bulk, in place) ----
            for X in (qT[0], qT[1], kT[0], kT[1]):
                tmp = osb.tile([128, T], MMDT, tag="rope_tmp", name=f"rtmp_{id(X) % 97}")
                # tmp = swap(X) (exchange 32-halves within each head)
                for h in range(2):
                    b0 = h * 64
                    nc.vector.tensor_copy(tmp[b0:b0 + 32, :], X[b0 + 32:b0 + 64, :])
                    nc.vector.tensor_copy(tmp[b0 + 32:b0 + 64, :], X[b0:b0 + 32, :])
                nc.vector.tensor_tensor(X[:], X[:], cos_sb[:], ALU.mult)
                nc.vector.tensor_tensor(tmp[:], tmp[:], sin_sb[:], ALU.mult)
                nc.vector.tensor_tensor(X[:], X[:], tmp[:], ALU.add)

            # ---- Phase B: causal attention ----
            for p in range(2):            # head pairs
                for qc in range(NTC):     # q-chunks
                    nk = 4 * qc + 4
                    yps = [ps.tile([D + 1, TCH], F32, tag="acc", name=f"yps_{p}_{qc}_{_h}") for _h in range(2)]
                    for ktp in range(nk // 2):   # k-tile pairs
                        sc = [ps.tile([128, 2 * TCH], F32, tag="sc2", name=f"sc_{p}_{qc}_{ktp}_{_h}")
                              for _h in range(2)]
                        et = [esb.tile([128, 2 * TCH], MMDT, tag="et", name=f"et_{p}_{qc}_{ktp}_{_h}")
                              for _h in range(2)]
                        for half in range(2):
                            kt = 2 * ktp + half
                            for h in range(2):
                                nc.tensor.matmul(
                                    sc[h][:, half * TCH:(half + 1) * TCH],
                                    kT[p][h * 64:(h + 1) * 64,
                                          kt * 128:(kt + 1) * 128],
                                    qT[p][h * 64:(h + 1) * 64,
                                          qc * TCH:(qc + 1) * TCH],
                                    start=True, stop=True,
                                    tile_position=(64 * h, 0))
                        for h in range(2):
                            nc.scalar.activation(et[h][:], sc[h][:], AF.Exp,
                                                 scale=0.125)
                        # causal masking for diagonal-crossing tiles
                        for half in range(2):
                            kt = 2 * ktp + half
                            m = kt - 4 * qc
                            if m >= 0:
                                w = 128 * (m + 1)
                                off = half * TCH
                                for h in range(2):
                                    nc.vector.tensor_tensor(
                                        et[h][:, off:off + w],
                                        et[h][:, off:off + w],
                                        mask_sb[m][:, 0:w], ALU.mult)
                        # attn @ v
                        for half in range(2):
                            kt = 2 * ktp + half
                            for h in range(2):
                                hh = 2 * p + h
                                nc.tensor.matmul(
                                    yps[h][:],
                                    vaug[kt][:, hh * (D + 1):(hh + 1) * (D + 1)],
                                    et[h][:, half * TCH:(half + 1) * TCH],
                                    start=(kt == 0), stop=(kt == nk - 1))
                    # normalize via denominator row
                    for h in range(2):
                        dln = osb.tile([1, TCH], F32, tag="dln", name=f"dln_{p}_{qc}_{h}")
                        drec = osb.tile([1, TCH], MMDT, tag="drec", name=f"drec_{p}_{qc}_{h}")
                        nc.scalar.activation(dln[:], yps[h][D:D + 1, :], AF.Ln)
                        nc.scalar.activation(drec[:], dln[:], AF.Exp, scale=-1.0)
                        pb = ps.tile([D, TCH], F32, tag="p512", name=f"pb_{p}_{qc}_{h}")
                        nc.tensor.matmul(pb[:], ones1_sb[:], drec[:],
                                         start=True, stop=True)
                        rb = osb.tile([D, TCH], MMDT, tag="rb", name=f"rb_{p}_{qc}_{h}")
                        nc.scalar.copy(rb[:], pb[:])
                        nc.vector.tensor_tensor(
                            yT[p][h * 64:(h + 1) * 64, qc * TCH:(qc + 1) * TCH],
                            yps[h][0:D, :], rb[:], ALU.mult)

            if _DEBUG_DUMPS:
                for src_t, dst_t in [(yT[1], dq_d), (kT[0], dk_d), (yT[0], dy_d)]:
                    dt_ = osb.tile([128, T], F32, tag="rope_tmp", name=f"dbgd_{dst_t.name}")
                    nc.vector.tensor_copy(dt_[:], src_t[:])
                    nc.sync.dma_start(out=dst_t[:], in_=dt_[:])
                dvt = osb.tile([128, HPC * (D + 1)], F32, tag="ot", name="dbgv", bufs=3)
                nc.vector.tensor_copy(dvt[:], vaug[0][:])
                nc.sync.dma_start(out=dv_d[:], in_=dvt[:])

            # ---- Phase C: output projection ----
            for tt in range(NTT):
                for nchunk in range(2):
                    pp = ps.tile([128, TCH], F32, tag="p512", name=f"pp_{tt}_{nchunk}")
                    for kk in range(2):
                        nc.tensor.matmul(
                            pp[:],
                            yT[kk][:, tt * 128:(tt + 1) * 128],
                            wp_sb[kk][:, nchunk * TCH:(nchunk + 1) * TCH],
                            start=(kk == 0), stop=(kk == 1))
                    ot = osb.tile([128, TCH], F32, tag="ot", name=f"ot_{tt}_{nchunk}", bufs=3)
                    nc.vector.tensor_copy(ot[:], pp[:])
                    nc.sync.dma_start(
                        out=out_d[tt * 128:(tt + 1) * 128,
                                  nchunk * TCH:(nchunk + 1) * TCH],
                        in_=ot[:])

    nc.finalize()
    return nc


def _rope_tables():
    dd = (np.arange(128) % 64) % 32
    fraction = (2.0 * np.arange(32, dtype=np.float32) / 64).astype(np.float32)
    timescale = (np.float32(10000.0) ** fraction).astype(np.float32)
    pos = np.arange(T, dtype=np.float32)
    ang = (pos[None, :] / timescale[dd][:, None]).astype(np.float32)  # [128, T]
    cos_t = np.cos(ang).astype(np.float32)
    sin_t = np.sin(ang).astype(np.float32)
    sgn = np.where((np.arange(128) % 64) < 32, np.float32(-1.0), np.float32(1.0))
    sin_signed = (sin_t * sgn[:, None]).astype(np.float32)
    return cos_t, sin_signed


def _mask_tiles():
    masks = np.zeros((4 * 128, TCH), np.float32)
    r = np.arange(128)[:, None]
    c = np.arange(TCH)[None, :]
    for m in range(4):
        masks[m * 128:(m + 1) * 128] = (c >= 128 * m + r).astype(np.float32)
    return masks


def kernel(x, W_attn, b_attn, W_proj, b_proj):
    x = np.asarray(x, np.float32)
    W_attn = np.asarray(W_attn, np.float32)
    b_attn = np.asarray(b_attn, np.float32)
    W_proj = np.asarray(W_proj, np.float32)
    b_proj = np.asarray(b_proj, np.float32)

    has_battn = bool(np.any(b_attn != 0))
    key = ("v4bf", has_battn)
    if key not in _prog_cache:
        _prog_cache[key] = _build_program(has_battn)
    nc = _prog_cache[key]

    import ml_dtypes
    bf = ml_dtypes.bfloat16
    cos_t, sin_signed = _rope_tables()
    masks = _mask_tiles().astype(bf)
    ones4 = np.ones((128, HPC), bf)
    ones1 = np.ones((1, D), bf)

    in_maps = []
    for core in range(NCORES):
        b, g = divmod(core, HPC)
        hs = [HPC * g + i for i in range(HPC)]
        qcols, kcols, vcols = [], [], []
        for i in range(0, HPC, 2):
            qcols += list(range(hs[i] * D, (hs[i] + 1) * D))
            qcols += list(range(hs[i + 1] * D, (hs[i + 1] + 1) * D))
        for i in range(0, HPC, 2):
            kcols += [C + cc for cc in range(hs[i] * D, (hs[i] + 1) * D)]
            kcols += [C + cc for cc in range(hs[i + 1] * D, (hs[i + 1] + 1) * D)]
        vcols = [2 * C + cc for h in hs for cc in range(h * D, (h + 1) * D)]
        rows = [h * D + d for h in hs for d in range(D)]

        in_maps.append({
            "xT": np.ascontiguousarray(x[b].T).astype(bf),
            "wqk": np.ascontiguousarray(W_attn[:, qcols + kcols]).astype(bf),
            "wv": np.ascontiguousarray(W_attn[:, vcols]).astype(bf),
            "wp": np.ascontiguousarray(W_proj[rows, :]).astype(bf),
            "cos_t": cos_t, "sin_t": sin_signed, "masks": masks,
            "ones4": ones4, "ones1": ones1,
            "bqk": np.ascontiguousarray(
                b_attn[qcols + kcols].reshape(-1, 1)),
            "vbias": np.tile(b_attn[vcols], (128, 1)).astype(bf),
        })

    trace = bool(os.environ.get("TRNK_TRACE"))
    if trace:
        try:
            import ntff_shim  # noqa: F401
        except ImportError:
            trace = False
    res = run_bass_kernel_spmd(nc, in_maps, list(range(NCORES)), trace=trace)
    if trace:
        globals()["_last_exec_time_ns"] = res.exec_time_ns
        globals()["_last_trace"] = res.instructions_and_trace
        globals()["_last_profile_json"] = res.profile_json

    out = np.zeros((B, T, C), np.float32)
    for core in range(NCORES):
        b = core // HPC
        out[b] += res.results[core]["out"]
    out += b_proj[None, None, :]
    return out

